# revision 1
# baseline (speedup 1.0000x reference)
"""Trainium2 Bass kernel for nn_DeepQNetwork (dense_mlp).

Reference computation (per row of x [B, 15]):
    keep = x[:, :11]
    hold_oh = one_hot(int(x[:, 11]), 4)
    nxt_oh  = one_hot(int(x[:, 12:15]) - 1, 7) each  -> 21 cols
    inp = [keep, hold_oh, nxt_oh]            # [B, 36]
    h1 = relu(inp @ W1 + b1)                 # [B, 128]
    h2 = relu(h1 @ W2 + b2)                  # [B, 512]
    out = h2 @ W4 + b4                       # [B, 40]

Strategy: pure data parallel over 8 NeuronCores (batch sharded, weights
replicated).  On-core dataflow is feature-major ([features, batch]) so all
matmuls chain on the TensorEngine:

  x tile [128b, 15f] --PE transpose--> psum XT [15, Bt]
  XT --copy--> SBUF rhs40 rows 0:15 (11 keep + 4 raw id rows)
  rhs40[0:15] --selection matmul E--> psum REP [25, Bt] (id rows replicated)
  REP --DVE is_equal vs per-partition class consts--> rhs40 rows 15:40 (one-hot)
  rhs40 --W1' (reordered W1 with zeros for raw id rows)--> h1 -> h2 -> out
  out [40, Bt] --PE transpose--> [128b, 40] -> DMA to DRAM

Matmuls run as float32r (fp32 storage, single-pass PE = 4x fp32
throughput, ~1e-4 relative error).  The input front (transpose / copy /
replicate / one-hot compare) runs SKEW supertiles ahead of compute into an
SBUF-resident rhs_all, PSUM tiles are single-bank with per-512-slice drain
ops split across ScalarE and VectorE, and output is PE-transposed back to
batch-major before a strided DMA store.
"""

import numpy as np

try:  # persistent XLA/NEFF cache: makes fresh-process compiles fast
    import jax as _jax

    _jax.config.update("jax_compilation_cache_dir", "/tmp/jax_neff_cache")
    _jax.config.update("jax_persistent_cache_min_compile_time_secs", 1.0)
except Exception:
    pass

import concourse.bacc as bacc
import concourse.bass as bass
import concourse.mybir as mybir
import concourse.tile as tile
from concourse.bass_utils import run_bass_kernel_spmd

N_CORES = 8
B_TOTAL = 131072
B_CORE = B_TOTAL // N_CORES  # 16384
BT = 1024                    # batch per supertile
N_ST = B_CORE // BT          # supertiles per core
E_CH = BT // 128             # 128-row transpose chunks per supertile (8)
N_SL = BT // 512             # 512-wide matmul slices per supertile (2)

F32 = mybir.dt.float32
F32R = mybir.dt.float32r

_BUILT = {}


def _mm_dt(ap, mode):
    return ap


def _build(mode: str):
    """Build the per-core Bass module (same NEFF on all 8 cores)."""
    nc = bacc.Bacc("TRN2", target_bir_lowering=False, debug=False)
    MMD = F32R if mode == "f32r" else F32

    x_d = nc.dram_tensor("x", [B_CORE, 15], MMD, kind="ExternalInput").ap()
    w1_d = nc.dram_tensor("w1p", [47, 128], MMD, kind="ExternalInput").ap()
    w2_d = nc.dram_tensor("w2p", [128, 512], MMD, kind="ExternalInput").ap()
    w4_d = nc.dram_tensor("w4p", [128, 160], MMD, kind="ExternalInput").ap()
    b1_d = nc.dram_tensor("b1p", [128, 1], F32, kind="ExternalInput").ap()
    b2_d = nc.dram_tensor("b2p", [128, 4], F32, kind="ExternalInput").ap()
    b4_d = nc.dram_tensor("b4p", [40, 1], F32, kind="ExternalInput").ap()
    e_d = nc.dram_tensor("esel", [15, 32], MMD, kind="ExternalInput").ap()
    cv_d = nc.dram_tensor("cvec", [32, 1], F32, kind="ExternalInput").ap()
    id_d = nc.dram_tensor("ident", [128, 128], MMD, kind="ExternalInput").ap()
    idf_d = nc.dram_tensor("identf", [40, 40], F32, kind="ExternalInput").ap()
    out_d = nc.dram_tensor("out", [B_CORE, 40], F32, kind="ExternalOutput").ap()

    from contextlib import ExitStack

    with tile.TileContext(nc) as tc, ExitStack() as ctx:
        consts = ctx.enter_context(tc.tile_pool(name="consts", bufs=1))
        w1_sb = consts.tile([47, 128], MMD, tag="w1")
        w2_sb = consts.tile([128, 512], MMD, tag="w2")
        w4_sb = consts.tile([128, 160], MMD, tag="w4")
        b1_sb = consts.tile([128, 1], F32, tag="b1")
        b2_sb = consts.tile([128, 4], F32, tag="b2")
        b4_sb = consts.tile([40, 1], F32, tag="b4")
        e_full = consts.tile([47, 32], MMD, tag="esel")
        e_sb = e_full[32:47, :]
        cv_sb = consts.tile([32, 1], F32, tag="cvec")
        id_sb = consts.tile([128, 128], MMD, tag="ident")
        idf_sb = consts.tile([40, 40], F32, tag="identf")
        nc.sync.dma_start(w1_sb[:], w1_d)
        nc.sync.dma_start(w2_sb[:], w2_d)
        nc.sync.dma_start(w4_sb[:], w4_d)
        nc.sync.dma_start(b1_sb[:], b1_d)
        nc.sync.dma_start(b2_sb[:], b2_d)
        nc.sync.dma_start(b4_sb[:], b4_d)
        nc.sync.dma_start(e_sb, e_d)
        nc.sync.dma_start(cv_sb[:], cv_d)
        nc.sync.dma_start(id_sb[:], id_d)
        nc.sync.dma_start(idf_sb[:], idf_d)

        sbuf = ctx.enter_context(tc.tile_pool(name="sbuf", bufs=4))
        xpool = ctx.enter_context(tc.tile_pool(name="xpool", bufs=4))
        rhs_pool = ctx.enter_context(tc.tile_pool(name="rhs_pool", bufs=1))
        rhs_all = rhs_pool.tile([47, B_CORE], MMD, tag="rhs_all")
        psum_io = ctx.enter_context(
            tc.tile_pool(name="psum_io", bufs=3, space="PSUM")
        )
        psum = ctx.enter_context(tc.tile_pool(name="psum", bufs=5, space="PSUM"))

        # per-supertile state threaded between pipeline phases
        S = {}

        def dma_x(st):
            if not (0 <= st < N_ST):
                return
            xv = x_d[st * BT:(st + 1) * BT, :].rearrange(
                "(p e) f -> p e f", p=128
            )
            x_sb = xpool.tile([128, E_CH, 15], MMD, tag="x", name=f"x{st}")
            nc.sync.dma_start(x_sb[:], xv)
            S[st] = {"x": x_sb}

        def in_transpose(st):
            # one psum tile for the whole input stage: rows 0:15 = x
            # transposed (feature-major), rows 32:64 = replicated id rows
            if not (0 <= st < N_ST):
                return
            d = S[st]
            d["p_xt"] = []
            for s in range(N_SL):
                p_in = psum_io.tile(
                    [15, 512], MMD, tag="io", name=f"in{st}_{s}"
                )
                d["p_xt"].append(p_in)
                for e in range(4):
                    nc.tensor.transpose(
                        p_in[:, e * 128:(e + 1) * 128],
                        d["x"][:, s * 4 + e, :], id_sb[:],
                    )

        def xt_copy(st):
            # rhs47 layout: rows 0:25 one-hot, 25:32 zero pad,
            # rows 32:43 keep features, 43:47 raw id values (hold, n0..n2)
            if not (0 <= st < N_ST):
                return
            d = S[st]
            d["rhs"] = rhs_all[:, st * BT:(st + 1) * BT]
            for s in range(N_SL):
                nc.scalar.copy(
                    d["rhs"][32:47, s * 512:(s + 1) * 512], d["p_xt"][s][:]
                )


        def rep_mm(st):
            # replicate id rows: REP[r, b] = v_{src(r)}(b)
            if not (0 <= st < N_ST):
                return
            d = S[st]
            d["p_rep"] = []
            for s in range(N_SL):
                sl = bass.ts(s, 512)
                p_rep = psum_io.tile([32, 512], F32, tag="io", name=f"rep{st}_{s}")
                d["p_rep"].append(p_rep)
                nc.tensor.matmul(
                    p_rep[:], e_sb, d["rhs"][32:47, sl],
                    start=True, stop=True,
                )

        def iseq(st):
            # one-hot: (v == class_const) per partition; pad rows compare
            # against -1 so they are always written 0.
            if not (0 <= st < N_ST):
                return
            d = S[st]
            for s in range(N_SL):
                nc.vector.tensor_scalar(
                    d["rhs"][0:32, s * 512:(s + 1) * 512], d["p_rep"][s][:],
                    cv_sb[:], None, op0=mybir.AluOpType.is_equal,
                )

        def l1(st):
            # L1: h1 = relu(W1'.T @ rhs47 + b1)  [128, BT], per 512-slice
            if not (0 <= st < N_ST):
                return
            d = S[st]
            d["h1"] = sbuf.tile([128, BT], MMD, tag="h1", name=f"h1s{st}")
            for s in range(N_SL):
                sl = bass.ts(s, 512)
                p_h1 = psum.tile([128, 512], F32, tag="big", name=f"h1{st}_{s}")
                nc.tensor.matmul(
                    p_h1[:], w1_sb[:], d["rhs"][:, sl],
                    start=True, stop=True,
                )
                nc.scalar.activation(
                    d["h1"][:, sl], p_h1[:], mybir.ActivationFunctionType.Relu,
                    bias=b1_sb[:], scale=1.0,
                )

        def l2(st, gs):
            # L2 chunks g: relu(W2[:, 128g:].T @ h1 + b2[g])  [128, BT]
            if not (0 <= st < N_ST):
                return
            d = S[st]
            h2 = d.setdefault("h2", {})
            for g in gs:
                h2g = sbuf.tile([128, BT], MMD, tag=f"h2_{g}", name=f"h2s{st}_{g}")
                for s in range(N_SL):
                    sl = bass.ts(s, 512)
                    p_h2 = psum.tile(
                        [128, 512], F32, tag="big", name=f"h2{st}_{g}_{s}"
                    )
                    nc.tensor.matmul(
                        p_h2[:], w2_sb[:, g * 128:(g + 1) * 128],
                        d["h1"][:, sl], start=True, stop=True,
                    )
                    if s == 0:
                        nc.scalar.activation(
                            h2g[:, sl], p_h2[:],
                            mybir.ActivationFunctionType.Relu,
                            bias=b2_sb[:, g:g + 1], scale=1.0,
                        )
                    else:
                        nc.vector.tensor_scalar(
                            h2g[:, sl], p_h2[:], b2_sb[:, g:g + 1], 0.0,
                            op0=mybir.AluOpType.add, op1=mybir.AluOpType.max,
                        )
                h2[g] = h2g

        def l3(st):
            # L3: out = W4.T @ h2 + b4   [40, BT], contract 512 in 4 chunks
            if not (0 <= st < N_ST):
                return
            d = S[st]
            d["o_sb"] = sbuf.tile([40, BT], F32, tag="osb", name=f"osb{st}")
            for s in range(N_SL):
                sl = bass.ts(s, 512)
                p_out = psum_io.tile([40, 512], F32, tag="io", name=f"out{st}_{s}")
                for c in range(4):
                    nc.tensor.matmul(
                        p_out[:], w4_sb[:, c * 40:(c + 1) * 40],
                        d["h2"][c][:, sl],
                        start=(c == 0), stop=(c == 3),
                    )
                nc.vector.tensor_scalar(
                    d["o_sb"][:, sl], p_out[:], b4_sb[:], None,
                    op0=mybir.AluOpType.add,
                )

        def out_t(st):
            # back to batch-major: transpose [40, 128] slices -> [128, 40]
            if not (0 <= st < N_ST):
                return
            d = S[st]
            p_ot = psum_io.tile([128, E_CH * 40], F32, tag="io", name=f"ot{st}")
            for t in range(E_CH):
                nc.tensor.transpose(
                    p_ot[:, t * 40:(t + 1) * 40],
                    d["o_sb"][:, t * 128:(t + 1) * 128],
                    idf_sb[:],
                )
            ot_sb = sbuf.tile([128, E_CH * 40], F32, tag="otsb", name=f"otsb{st}")
            nc.scalar.copy(ot_sb[:], p_ot[:])

            ov = out_d[st * BT:(st + 1) * BT, :].rearrange(
                "(p e) f -> p e f", p=128
            )
            src = ot_sb[:].rearrange("p (e k) -> p e k", k=40)
            nc.sync.dma_start(ov, src)
            del S[st]

        # Software pipeline with the input front (transpose/copy/rep/
        # one-hot into rhs_all) running SKEW supertiles ahead of compute.
        # Emission order fixes each engine's FIFO; the deep skew gives
        # input ops slack so compute never waits on them.
        SKEW = 3
        dma_x(0)
        dma_x(1)
        for st in range(SKEW):
            in_transpose(st)
            xt_copy(st)
            rep_mm(st)
            iseq(st)
            dma_x(st + 2)
        l1(0)
        for st in range(N_ST):
            l2(st, [0])
            out_t(st - 2)
            l2(st, [1])
            in_transpose(st + SKEW)
            xt_copy(st + SKEW)
            l2(st, [2])
            l2(st, [3])
            rep_mm(st + SKEW)
            iseq(st + SKEW)
            l1(st + 1)
            l3(st)
            dma_x(st + SKEW + 2)
        out_t(N_ST - 2)
        out_t(N_ST - 1)

    nc.compile()
    return nc


def _prep_inputs(x, W1, b1, W2, b2, W4, b4):
    """Host-side packing of weights into DMA/layout-friendly tensors."""
    f = np.float32
    x, W1, b1, W2, b2, W4, b4 = (
        np.asarray(a) for a in (x, W1, b1, W2, b2, W4, b4)
    )
    w1p = np.concatenate(
        [W1[11:36], np.zeros((7, 128), f), W1[0:11], np.zeros((4, 128), f)],
        axis=0,
    ).astype(f)  # [47, 128]: one-hot wts, pad, keep wts, raw-id zeros
    w2p = np.ascontiguousarray(W2.astype(f))  # [128, 512]
    # [512, 40] -> chunks c of 128 rows side by side -> [128, 4*40]
    w4p = np.ascontiguousarray(
        W4.astype(f).reshape(4, 128, 40).transpose(1, 0, 2).reshape(128, 160)
    )
    b1p = b1.astype(f).reshape(128, 1)
    b2p = np.ascontiguousarray(b2.astype(f).reshape(4, 128).T)  # [128, 4]
    b4p = b4.astype(f).reshape(40, 1)
    # selection matrix: rows = source feature (0..14), cols = replicated row
    esel = np.zeros((15, 32), f)
    esel[11, 0:4] = 1.0        # hold -> 4 classes
    esel[12, 4:11] = 1.0       # next0 -> 7 classes
    esel[13, 11:18] = 1.0
    esel[14, 18:25] = 1.0
    cvec = np.concatenate(
        [np.arange(4), np.arange(1, 8), np.arange(1, 8), np.arange(1, 8),
         np.full(7, -1.0)]
    ).astype(f).reshape(32, 1)
    ident = np.eye(128, dtype=f)
    identf = np.eye(40, dtype=f)
    shared = dict(w1p=w1p, w2p=w2p, w4p=w4p, b1p=b1p, b2p=b2p, b4p=b4p,
                  esel=esel, cvec=cvec, ident=ident, identf=identf)
    xs = np.ascontiguousarray(x.astype(f))
    in_maps = []
    for c in range(N_CORES):
        m = dict(shared)
        m["x"] = np.ascontiguousarray(xs[c * B_CORE:(c + 1) * B_CORE])
        in_maps.append(m)
    return in_maps


def _get_nc(mode):
    if mode not in _BUILT:
        _BUILT[mode] = _build(mode)
    return _BUILT[mode]


def run(x, W1, b1, W2, b2, W4, b4, mode="f32r", **kw):
    nc = _get_nc(mode)
    in_maps = _prep_inputs(x, W1, b1, W2, b2, W4, b4)
    res = run_bass_kernel_spmd(nc, in_maps, core_ids=list(range(N_CORES)), **kw)
    out = np.concatenate([r["out"] for r in res.results], axis=0)
    return out, res


def kernel(x, W1, b1, W2, b2, W4, b4):
    out, _ = run(x, W1, b1, W2, b2, W4, b4)
    return out



# revision 22
# speedup vs baseline: 1.7198x; 1.7198x over previous
"""Trainium2 Bass kernel for nn_DeepQNetwork (dense_mlp).

Reference computation (per row of x [B, 15]):
    keep = x[:, :11]
    hold_oh = one_hot(int(x[:, 11]), 4)
    nxt_oh  = one_hot(int(x[:, 12:15]) - 1, 7) each  -> 21 cols
    inp = [keep, hold_oh, nxt_oh]            # [B, 36]
    h1 = relu(inp @ W1 + b1)                 # [B, 128]
    h2 = relu(h1 @ W2 + b2)                  # [B, 512]
    out = h2 @ W4 + b4                       # [B, 40]

Strategy: pure data parallel over 8 NeuronCores (batch sharded, weights
replicated).  Host-side prep is layout-only: x is cast to bf16,
transposed to feature-major [36, B_CORE] with the id columns
pre-replicated (hold x4, next x7 each) so the device does NO transposes
and NO replication matmul.  On-device per 1024-column supertile:

  iseq   DVE   in-place is_equal vs per-partition class consts turns the
               replicated id rows 11:36 into exact one-hot (bf16)
  L1     PE    h1 = relu(W1.T @ xT + b1)      2x ap512 matmul, Act drain
  L2     PE    h2 = relu(W2.T @ h1 + b2)      8x ap512 matmul; drains
               split across Act / DVE / GpSimd
  L3     PE    out = W4.T @ h2 + b4 done BATCH-MAJOR: h2 128-col blocks
               are the stationary operand, W4 chunks [128,40] stream ->
               psum [128b, 40] accumulated over 4 K-chunks (32 tiny
               ap40 matmuls, 16.7ns each).  Output lands batch-major, so
               no output transpose; DVE adds b4 while draining to SBUF.

All matmuls bf16 (1 cycle/row; fp32r would be 4x slower at ap=40).
The xT batch order is permuted host-side so each SBUF partition holds 8
consecutive output rows, making the store DMA 1280B-contiguous.
Software pipeline: L1 runs 1 supertile ahead, L3 one behind, so PE never
waits on activation drains.
"""

import numpy as np

try:  # persistent XLA/NEFF cache: makes fresh-process compiles fast
    import jax as _jax

    _jax.config.update("jax_compilation_cache_dir", "/tmp/jax_neff_cache")
    _jax.config.update("jax_persistent_cache_min_compile_time_secs", 1.0)
except Exception:
    pass

import ml_dtypes

import concourse.bacc as bacc
import concourse.bass as bass
import concourse.mybir as mybir
import concourse.tile as tile
from concourse.bass_utils import run_bass_kernel_spmd

N_CORES = 8
B_TOTAL = 131072
B_CORE = B_TOTAL // N_CORES  # 16384
BT = 1024                    # batch columns per supertile
N_ST = B_CORE // BT          # supertiles per core (16)
XCH = 2                      # supertiles per x-load DMA

F32 = mybir.dt.float32
BF16 = mybir.dt.bfloat16
BF16_NP = ml_dtypes.bfloat16

_BUILT = {}


def _build(mode: str = "bf16"):
    """Build the per-core Bass module (same NEFF on all 8 cores)."""
    nc = bacc.Bacc("TRN2", target_bir_lowering=False, debug=False)

    xt_d = nc.dram_tensor("xt", [36, B_CORE], BF16, kind="ExternalInput").ap()
    # all constants packed into two DMAs (one per dtype): cuts serialized
    # HWDGE issue latency out of the pipeline fill.  cvec rides in the bf16
    # pack (ids are exact in bf16) so the f32 biases can land later.
    # cb layout: [w1 | w2 | w4] = [128, 128+512+160]
    # cf layout: [b1 | b2 | b4t | cvec] = [128, 1+4+320+1]
    cb_d = nc.dram_tensor("cb", [128, 800], BF16, kind="ExternalInput").ap()
    cf_d = nc.dram_tensor("cf", [128, 326], F32, kind="ExternalInput").ap()
    out_d = nc.dram_tensor("out", [B_CORE, 40], F32, kind="ExternalOutput").ap()

    from contextlib import ExitStack

    with tile.TileContext(nc) as tc, ExitStack() as ctx:
        consts = ctx.enter_context(tc.tile_pool(name="consts", bufs=1))
        cb_sb = consts.tile([128, 800], BF16, tag="cb")
        cf_sb = consts.tile([128, 326], F32, tag="cf")
        w1_sb = cb_sb[0:36, 0:128]
        b1_sb = cf_sb[:, 0:1]
        b4_sb = cf_sb[:, 5:325]
        cv_sb = cf_sb[0:25, 325:326]

        xpool = ctx.enter_context(tc.tile_pool(name="xpool", bufs=1))
        xt_sb = xpool.tile([36, B_CORE], BF16, tag="xt")

        sbuf = ctx.enter_context(tc.tile_pool(name="sbuf", bufs=2))
        opool = ctx.enter_context(tc.tile_pool(name="opool", bufs=3))
        # two 2-buffer psum rotations (4 banks each = all 8 banks):
        # tagA {l1, g1, g3} drained by Act, tagD {g0, g2, l3} drained by DVE
        pla = ctx.enter_context(tc.tile_pool(name="pla", bufs=2, space="PSUM"))
        pld = ctx.enter_context(tc.tile_pool(name="pld", bufs=2, space="PSUM"))

        def dma_x(lo, hi):
            lo, hi = max(lo, 0), min(hi, N_ST)
            if lo >= hi:
                return
            sl = slice(lo * BT, hi * BT)
            nc.sync.dma_start(xt_sb[:, sl], xt_d[:, sl])

        # fill: cb issues from the (idle) Act queue in parallel with x0/cf
        # on SP; cf (f32 biases) is first needed only by the L1 drain.
        # cvec is a structural constant (one-hot class ids): build it with
        # memsets on the idle DVE instead of waiting on a DMA.
        dma_x(0, 1)
        nc.scalar.dma_start(cb_sb[:], cb_d)
        nc.sync.dma_start(cf_sb[:], cf_d)
        dma_x(1, 2)
        dma_x(2, 4)
        dma_x(4, 8)
        dma_x(8, 12)
        dma_x(12, 16)

        # per-supertile state threaded between pipeline phases
        S = {i: {} for i in range(N_ST)}

        def iseq(st, eng="pool"):
            # one-hot in place: replicated id rows 0:25 -> (v == class_c)
            # on GpSimd: the only SBUF-only op, freeing Act/DVE for drains
            if not (0 <= st < N_ST):
                return
            sl = bass.ts(st, BT)
            e = nc.gpsimd if eng == "pool" else nc.vector
            e.tensor_scalar(
                xt_sb[0:25, sl], xt_sb[0:25, sl], cv_sb, None,
                op0=mybir.AluOpType.is_equal,
            )

        def l1(st):
            # h1 = relu(W1.T @ xT + b1)  [128, BT]
            if not (0 <= st < N_ST):
                return
            d = S[st]
            p = pla.tile([128, BT], F32, tag="A", name=f"pl1_{st}")
            for s in range(2):
                nc.tensor.matmul(
                    p[:, s * 512:(s + 1) * 512], w1_sb,
                    xt_sb[:, st * BT + s * 512: st * BT + (s + 1) * 512],
                    start=True, stop=True,
                )
            d["h1"] = sbuf.tile([128, BT], BF16, tag="h1", name=f"h1_{st}")
            nc.scalar.activation(
                d["h1"][:], p[:], mybir.ActivationFunctionType.Relu,
                bias=b1_sb, scale=1.0,
            )

        def l2_mm(st, g):
            if not (0 <= st < N_ST):
                return
            d = S[st]
            pool_ = pld if g in (0, 2) else pla
            tag = "D" if g in (0, 2) else "A"
            p = pool_.tile([128, BT], F32, tag=tag, name=f"pl2_{st}_{g}")
            d[f"p2_{g}"] = p
            for s in range(2):
                sl = bass.ts(s, 512)
                nc.tensor.matmul(
                    p[:, sl], cb_sb[:, 128 + g * 128:128 + (g + 1) * 128],
                    d["h1"][:, sl], start=True, stop=True,
                )

        def l2_drain(st, g, eng, lo=0, hi=BT):
            # h2_g = relu(p + b2[g]); engine-split to keep PE the bottleneck
            if not (0 <= st < N_ST):
                return
            d = S[st]
            if f"h2_{g}" not in d:
                d[f"h2_{g}"] = sbuf.tile(
                    [128, BT], BF16, tag=f"h2_{g}", name=f"h2_{st}_{g}"
                )
            h2 = d[f"h2_{g}"]
            p = d[f"p2_{g}"]
            if eng == "act":
                nc.scalar.activation(
                    h2[:, lo:hi], p[:, lo:hi],
                    mybir.ActivationFunctionType.Relu,
                    bias=cf_sb[:, 1 + g:2 + g], scale=1.0,
                )
            else:
                e = nc.vector if eng == "dve" else nc.gpsimd
                e.tensor_scalar(
                    h2[:, lo:hi], p[:, lo:hi], cf_sb[:, 1 + g:2 + g], 0.0,
                    op0=mybir.AluOpType.add, op1=mybir.AluOpType.max,
                )

        def l3(st, halves=1):
            # batch-major: psum[128b, 40] per 128-col block, accum 4 K-chunks
            if not (0 <= st < N_ST):
                return
            d = S[st]
            p = pld.tile([128, BT], F32, tag="D", name=f"pl3_{st}")
            osb = opool.tile([128, 320], F32, tag="osb", name=f"osb_{st}")
            d["osb"] = osb
            bph = 8 // halves
            for h in range(halves):
                for blk in range(h * bph, (h + 1) * bph):
                    po = p[:, blk * 40:(blk + 1) * 40]
                    for c in range(4):
                        nc.tensor.matmul(
                            po, d[f"h2_{c}"][:, blk * 128:(blk + 1) * 128],
                            cb_sb[:, 640 + c * 40:640 + (c + 1) * 40],
                            start=(c == 0), stop=(c == 3),
                        )
                lo, hi = h * bph * 40, (h + 1) * bph * 40
                nc.vector.tensor_tensor(
                    osb[:, lo:hi], p[:, lo:hi], b4_sb[:, lo:hi],
                    op=mybir.AluOpType.add,
                )
                if halves > 1:
                    dma_out_part(st, h * bph, (h + 1) * bph)

        def dma_out_part(st, blo, bhi):
            ov = out_d[st * BT:(st + 1) * BT, :].rearrange(
                "(p e) f -> p e f", p=128
            )[:, blo:bhi, :]
            src = S[st]["osb"][:, blo * 40:bhi * 40].rearrange(
                "p (e k) -> p e k", k=40
            )
            nc.sync.dma_start(ov, src)

        def dma_out(st, done=False):
            if not (0 <= st < N_ST):
                return
            if not done:
                dma_out_part(st, 0, 8)
            S.pop(st)

        # ---- software pipeline ----
        # PE stream per iteration: [L1(st+1) g0 g1 g2 g3 L3(st-1)].
        # Act drain queue [L1d, g1d, g3d] and DVE queue [g0d, g2d, L3d]
        # are each gated in increasing PE-completion order, and each psum
        # rotation (tagA/tagD) is reused a full iteration later.
        iseq(0, "dve")
        iseq(1, "dve")
        l1(0)
        for st in range(N_ST + 1):
            iseq(st + 2)
            l1(st + 1)
            l2_mm(st, 0)
            l2_drain(st, 0, "dve")
            l2_mm(st, 1)
            l2_drain(st, 1, "act")
            l2_mm(st, 2)
            l2_drain(st, 2, "dve")
            l2_mm(st, 3)
            l2_drain(st, 3, "act")
            l3(st - 1)
            dma_out(st - 1)

    nc.compile()
    return nc


# xT row layout: rows 0:4 hold x4, 4:25 next x7x3, 25:36 keep features
# (one-hot rows first: DVE partition ranges must start at partition 0)
_COL_IDX = [11] * 4 + [12] * 7 + [13] * 7 + [14] * 7 + list(range(11))
_W1_PERM = list(range(11, 36)) + list(range(0, 11))
_CVEC = np.concatenate(
    [np.arange(4), np.arange(1, 8), np.arange(1, 8), np.arange(1, 8)]
).astype(np.float32).reshape(25, 1)


def _prep_inputs(x, W1, b1, W2, b2, W4, b4):
    """Host-side packing: layout + dtype only (cast, transpose, replicate)."""
    f = np.float32
    x, W1, b1, W2, b2, W4, b4 = (
        np.asarray(a) for a in (x, W1, b1, W2, b2, W4, b4)
    )
    cb = np.zeros((128, 800), BF16_NP)
    cb[0:36, 0:128] = W1.astype(BF16_NP)[_W1_PERM]
    cb[:, 128:640] = W2.astype(BF16_NP)
    # [512, 40] -> K-chunks c of 128 rows side by side -> [128, 4*40]
    cb[:, 640:800] = (
        W4.astype(BF16_NP).reshape(4, 128, 40).transpose(1, 0, 2).reshape(128, 160)
    )
    cf = np.zeros((128, 326), f)
    cf[:, 0] = b1.astype(f)
    cf[:, 1:5] = b2.astype(f).reshape(4, 128).T
    cf[:, 5:325] = np.tile(b4.astype(f), (128, 8))
    cf[0:25, 325] = _CVEC[:, 0]
    shared = dict(cb=cb, cf=cf)
    xb = x.astype(BF16_NP)[:, _COL_IDX]  # [B, 36] replicated id cols
    in_maps = []
    for c in range(N_CORES):
        xc = xb[c * B_CORE:(c + 1) * B_CORE]
        # batch permutation: xT column st*1024 + e*128 + p holds batch row
        # st*1024 + 8p + e, so partition p stores 8 consecutive out rows
        xp = xc.reshape(N_ST, 128, 8, 36).transpose(0, 2, 1, 3)
        xt = np.ascontiguousarray(xp.reshape(B_CORE, 36).T)  # [36, B_CORE]
        m = dict(shared)
        m["xt"] = xt
        in_maps.append(m)
    return in_maps


def _get_nc(mode="bf16"):
    if mode not in _BUILT:
        _BUILT[mode] = _build(mode)
    return _BUILT[mode]


def run(x, W1, b1, W2, b2, W4, b4, mode="bf16", **kw):
    nc = _get_nc(mode)
    in_maps = _prep_inputs(x, W1, b1, W2, b2, W4, b4)
    res = run_bass_kernel_spmd(nc, in_maps, core_ids=list(range(N_CORES)), **kw)
    out = np.concatenate([r["out"] for r in res.results], axis=0)
    return out, res


def kernel(x, W1, b1, W2, b2, W4, b4):
    out, _ = run(x, W1, b1, W2, b2, W4, b4)
    return out
